# revision 13
# baseline (speedup 1.0000x reference)
"""Trainium2 Bass kernel: multi-head attention (B=4, T=2048, D=768, H=12).

Sharding: 8 cores = 4 batches x 2 head-groups (6 heads each).
Each core computes QKV projection (its heads), attention, and a partial
output projection (contraction over its 384 of 768 w_out rows).
Host unshard: out[b] = partial[2b] + partial[2b+1] (bias folded on core g=0).

Per-core dataflow (everything stays on-chip between input DMA and output DMA):
  - host supplies x[b] pre-transposed: xt [768, 2048]
  - Q^T/K^T computed in transposed layout (pair-packed [128, T] tiles);
    K^T stored per-head zero-padded so every matmul runs in 128-row mode
  - V computed in natural layout with a ones column per head (V~ [tok, 65])
    so the P@V matmul also produces softmax denominators
  - attention in S^T layout: S^T = K^T.T @ Q^T, P^T = exp(S^T/8) via ScalarE
    (fused PSUM eviction), attnU^T = V~.T @ P^T accumulated in PSUM.
    No max-subtraction (scores for this input distribution are within ~[-2.5, 2.8])
    and no transposes of P anywhere.
  - normalize: reciprocal of denominator row into a persistent zero-padded
    row tile, broadcast to 64 partitions with a rank-1 matmul against
    ones_pad (row 0 ones, rest zeros), multiply on DVE into out-proj lhsT layout
  - out-proj from attnN^T pair tiles; b_out via the zero-padded ones k-tile
  - matmuls run in float32r (1 cycle/row at N>=256 vs 4 for plain fp32;
    measured ~1.5e-4 matmul rel err vs 2.4e-3 for bf16). float32r operands
    must come from producers typed float32r, and memset cannot write f32r,
    so constant fills go through f32 twins + DVE copies.

This walrus build encodes at most one sync wait per instruction; Tile emits
several. _split_multi_waits() rewrites the final module, hoisting extra waits
onto same-engine nops inserted right before the offending instruction.
"""

import numpy as np

import concourse.bass as bass
import concourse.mybir as mybir
from concourse.tile import TileContext
from concourse.bass_utils import run_bass_kernel_spmd

# problem constants (fixed by the graded nn.Module)
B, T, D = 4, 2048, 768
H, HD = 12, 64
NCORES = 8
HL = H // 2            # heads per core (2 head-groups)
NPAIR = HL // 2        # head pairs per core

F32 = mybir.dt.float32
F32R = mybir.dt.float32r


def _patch_tile_drain():
    """Kept for API compatibility; the real fix is _split_multi_waits."""


def _split_multi_waits(nc):
    """Walrus here encodes only one sync wait per instruction. Move extra
    waits onto same-engine nops placed immediately before the instruction."""
    n = 0
    for f in nc.m.functions:
        for bb in f.blocks:
            new = []
            for inst in bb.instructions:
                si = inst.sync_info
                if si is not None and si.on_wait and len(si.on_wait) > 1:
                    extra = list(si.on_wait[:-1])
                    keep = si.on_wait[-1]
                    del si.on_wait[:]
                    si.on_wait.append(keep)
                    for w in extra:
                        nop = mybir.InstNoOp(name=f"I-wsplit-{n}", ins=[], outs=[])
                        n += 1
                        nop.engine = inst.engine
                        nop.sync_info = mybir.SyncInfo(on_wait=[w], on_update=[])
                        new.append(nop)
                new.append(inst)
            bb.instructions[:] = new
    return n


def build_nc(t=T, qc=1024, nch=512):
    """Build the SPMD per-core program. t = sequence length, qc = attention
    query chunk (PSUM-limited), nch = matmul moving-dim chunk."""
    tokt = t // 128            # token tiles
    nqc = t // qc              # query chunks
    dk = D // 128              # contraction tiles over D

    nc = bass.Bass("TRN2", target_bir_lowering=False, debug=False)

    # f32r-typed DRAM inputs: the host's fp32 bits reinterpret fine and the
    # PE rounds internally; this satisfies walrus's f32r provenance check.
    xt_d = nc.dram_tensor("xt", [D, t], F32R, kind="ExternalInput")
    wqk_d = nc.dram_tensor("wqk", [D, 2 * HL * HD], F32R, kind="ExternalInput")
    bqk_d = nc.dram_tensor("bqk", [128, 2 * HL * HD // 128], F32, kind="ExternalInput")
    wv_d = nc.dram_tensor("wv", [D + 1, HL * HD], F32R, kind="ExternalInput")
    wo_d = nc.dram_tensor("wo", [HL * HD + 1, D], F32R, kind="ExternalInput")
    out_d = nc.dram_tensor("out", [t, D], F32, kind="ExternalOutput")

    nmt = 2 * HL * HD // 128   # QK projection M-tiles (6)

    def MM(out, lhsT, rhs, start, stop):
        nc.tensor.matmul(out, lhsT, rhs, start=start, stop=stop)

    with TileContext(nc) as tc:
        lp = nc.allow_low_precision(reason="float32r matmul operand production")
        lp.__enter__()
        with tc.tile_pool(name="persist", bufs=1) as pp:
            # ones_pad: row 0 = 1.0, rows 1:128 = 0.0 (bias k-tile / broadcast lhsT)
            ones_pad = pp.tile([128, t], F32R, name="ones_pad")
            QT = [pp.tile([128, t], F32R, name=f"qt{p}") for p in range(NPAIR)]
            KT = [pp.tile([128, t], F32R, name=f"kt{h}") for h in range(HL)]
            V6 = [pp.tile([128, HL * (HD + 1)], F32R, name=f"v6_{c}") for c in range(tokt)]
            WOb = pp.tile([128, D], F32R, name="wob")
            bqk_t = pp.tile([128, nmt], F32, name="bqk_t")
            nc.sync.dma_start(out=bqk_t[:], in_=bqk_d[:, :])

            # ---------------- phase 1: projections ----------------
            with tc.tile_pool(name="phase1", bufs=1) as p1:
                # f32 constant sources (memset can't write f32r)
                ones32 = p1.tile([128, t], F32, name="ones32")
                nc.vector.memset(ones32[:], 0.0)
                nc.vector.memset(ones32[0:1, :], 1.0)
                nc.vector.tensor_copy(ones_pad[:], ones32[:])
                onesall32 = p1.tile([128, 8], F32, name="onesall32")
                nc.vector.memset(onesall32[:], 1.0)
                onesall = p1.tile([128, 8], F32R, name="onesall")
                nc.vector.tensor_copy(onesall[:], onesall32[:])
                z32 = p1.tile([128, D], F32, name="z32")
                nc.vector.memset(z32[:], 0.0)

                nc.vector.tensor_copy(WOb[:], z32[:])
                nc.sync.dma_start(out=WOb[0:1, :], in_=wo_d[HL * HD : HL * HD + 1, :])

                # zero-pad the complement half of each per-head K^T tile
                # (rows 64:128 of ones32 are zeros; 64-aligned partition ranges)
                for h in range(HL):
                    if h % 2 == 0:
                        nc.vector.tensor_copy(KT[h][64:128, :], ones32[64:128, :])
                    else:
                        nc.vector.tensor_copy(KT[h][0:64, :], ones32[64:128, :])

                xt_t = [p1.tile([128, t], F32R, name=f"x{k}") for k in range(dk)]
                for k in range(dk):
                    nc.sync.dma_start(out=xt_t[k][:], in_=xt_d[k * 128 : (k + 1) * 128, :])
                wqk_t = [p1.tile([128, 2 * HL * HD], F32R, name=f"wqk{k}") for k in range(dk)]
                for k in range(dk):
                    nc.sync.dma_start(
                        out=wqk_t[k][:], in_=wqk_d[k * 128 : (k + 1) * 128, :]
                    )
                wv_t = [p1.tile([128, HL * HD], F32R, name=f"wv{k}") for k in range(dk)]
                for k in range(dk):
                    nc.sync.dma_start(out=wv_t[k][:], in_=wv_d[k * 128 : (k + 1) * 128, :])
                wvb = p1.tile([128, HL * HD], F32R, name="wvb")
                nc.vector.tensor_copy(wvb[:], z32[:, 0 : HL * HD])
                nc.sync.dma_start(out=wvb[0:1, :], in_=wv_d[D : D + 1, :])

                with tc.tile_pool(name="psum_proj", bufs=2, space="PSUM") as prj:
                    # QK^T projection: M-tile 2p = q-pair p, 2p+1 = k-pair p
                    for p in range(NPAIR):
                        for m in (2 * p, 2 * p + 1):
                            for c in range(t // nch):
                                ps = prj.tile([128, nch], F32, tag="qk", bufs=2, name="psqk")
                                for k in range(dk):
                                    MM(
                                        ps[:],
                                        wqk_t[k][:, m * 128 : (m + 1) * 128],
                                        xt_t[k][:, c * nch : (c + 1) * nch],
                                        start=(k == 0),
                                        stop=(k == dk - 1),
                                    )
                                sl = slice(c * nch, (c + 1) * nch)
                                if m == 2 * p:
                                    nc.vector.tensor_scalar_add(
                                        QT[p][:, sl], ps[:], bqk_t[:, m : m + 1]
                                    )
                                else:
                                    h0, h1 = 2 * p, 2 * p + 1
                                    nc.vector.tensor_scalar_add(
                                        KT[h0][0:64, sl], ps[0:64, :], bqk_t[0:64, m : m + 1]
                                    )
                                    nc.vector.tensor_scalar_add(
                                        KT[h1][64:128, sl], ps[64:128, :], bqk_t[64:128, m : m + 1]
                                    )
                    # V projection (natural layout, scattered into V~ tiles)
                    for c in range(tokt):
                        tsl = slice(c * 128, (c + 1) * 128)
                        psv = prj.tile([128, HL * HD], F32, tag="v", bufs=2, name="psv")
                        for k in range(dk):
                            MM(
                                psv[:],
                                xt_t[k][:, tsl],
                                wv_t[k][:],
                                start=(k == 0),
                                stop=False,
                            )
                        MM(psv[:], ones_pad[:, tsl], wvb[:], start=False, stop=True)
                        v = V6[c]
                        v3 = v[:].rearrange("p (h c) -> p h c", c=HD + 1)
                        nc.vector.tensor_copy(
                            v3[:, :, HD : HD + 1],
                            onesall[:, 0:HL].rearrange("p (h c) -> p h c", c=1),
                        )
                        nc.vector.tensor_copy(
                            v3[:, :, 0:HD],
                            psv[:].rearrange("p (h c) -> p h c", c=HD),
                        )

            # ---------------- phases 2+3 ----------------
            with tc.tile_pool(name="persist2", bufs=1) as pp2:
                AN = [pp2.tile([128, t], F32R, name=f"an{p}") for p in range(NPAIR)]
                WO = [pp2.tile([128, D], F32R, name=f"wop{p}") for p in range(NPAIR)]
                for p in range(NPAIR):
                    nc.sync.dma_start(out=WO[p][:], in_=wo_d[p * 128 : (p + 1) * 128, :])
                # persistent reciprocal row: rows 1:128 zeroed exactly once
                r_pad = pp2.tile([128, qc], F32R, name="r_pad")
                zr32 = pp2.tile([128, qc], F32, name="zr32")
                nc.vector.memset(zr32[:], 0.0)
                nc.vector.tensor_copy(r_pad[:], zr32[:])

                # ---------------- phase 2: attention ----------------
                with (
                    tc.tile_pool(name="psum_s", bufs=2, space="PSUM") as s_pool,
                    tc.tile_pool(name="psum_u", bufs=2, space="PSUM") as u_pool,
                    tc.tile_pool(name="ptp", bufs=3) as ptp,
                ):
                    for p in range(NPAIR):
                        for j in range(2):
                            h = 2 * p + j
                            for q in range(nqc):
                                qsl = slice(q * qc, (q + 1) * qc)
                                au = u_pool.tile([65, qc], F32, tag="au", bufs=2, name="au")
                                for kb in range(tokt):
                                    st = s_pool.tile([128, qc], F32, tag="st", bufs=2, name="st")
                                    for c in range(qc // nch):
                                        MM(
                                            st[:, c * nch : (c + 1) * nch],
                                            KT[h][:, kb * 128 : (kb + 1) * 128],
                                            QT[p][:, q * qc + c * nch : q * qc + (c + 1) * nch],
                                            start=True,
                                            stop=True,
                                        )
                                    pt = ptp.tile([128, qc], F32R, tag="pt", bufs=3, name="pt")
                                    nc.scalar.activation(
                                        pt[:], st[:], mybir.ActivationFunctionType.Exp, scale=0.125
                                    )
                                    for c in range(qc // nch):
                                        MM(
                                            au[:, c * nch : (c + 1) * nch],
                                            V6[kb][:, h * (HD + 1) : (h + 1) * (HD + 1)],
                                            pt[:, c * nch : (c + 1) * nch],
                                            start=(kb == 0),
                                            stop=(kb == tokt - 1),
                                        )
                                # normalize: r = 1/denom into row 0 of r_pad,
                                # rank-1 broadcast matmul, multiply into AN
                                nc.vector.reciprocal(r_pad[0:1, :], au[64:65, :])
                                R = s_pool.tile([128, qc], F32, tag="st", bufs=2, name="R")
                                for c in range(qc // nch):
                                    MM(
                                        R[:, c * nch : (c + 1) * nch],
                                        ones_pad[:, 0:128],
                                        r_pad[:, c * nch : (c + 1) * nch],
                                        start=True,
                                        stop=True,
                                    )
                                R_sb = ptp.tile([64, qc], F32, tag="rsb", bufs=2, name="R_sb")
                                nc.vector.tensor_copy(R_sb[:], R[0:64, :])
                                nc.vector.tensor_mul(
                                    AN[p][j * 64 : (j + 1) * 64, qsl], au[0:64, :], R_sb[:]
                                )

                # ---------------- phase 3: output projection ----------------
                with (
                    tc.tile_pool(name="psum_o", bufs=2, space="PSUM") as o_pool,
                    tc.tile_pool(name="ost", bufs=3) as osp,
                ):
                    for c in range(tokt):
                        tsl = slice(c * 128, (c + 1) * 128)
                        ps = o_pool.tile([128, D], F32, tag="o", bufs=2, name="pso")
                        for n0 in range(0, D, nch):
                            nsz = min(nch, D - n0)
                            nsl = slice(n0, n0 + nsz)
                            for p in range(NPAIR):
                                MM(
                                    ps[:, nsl],
                                    AN[p][:, tsl],
                                    WO[p][:, nsl],
                                    start=(p == 0),
                                    stop=False,
                                )
                            MM(ps[:, nsl], ones_pad[:, tsl], WOb[:, nsl], start=False, stop=True)
                        so = osp.tile([128, D], F32, tag="so", bufs=3, name="so")
                        nc.vector.tensor_copy(so[:], ps[:])
                        nc.sync.dma_start(out=out_d[tsl, :], in_=so[:])
        lp.__exit__(None, None, None)

    return nc


def shard_inputs(x, w_qkv, b_qkv, w_out, b_out, t=T):
    """Build the 8 per-core input maps. Core = (batch, head-group)."""
    in_maps = []
    for core in range(NCORES):
        b, g = divmod(core, 2)
        hbase = HL * g * HD          # first qk column of this group (384*g)
        # q cols then k cols, pair-interleaved: M-tile 2p = q of heads (2p,2p+1),
        # M-tile 2p+1 = k of the same heads.
        wqk = np.empty((D, 2 * HL * HD), dtype=np.float32)
        bqk = np.empty((2 * HL * HD,), dtype=np.float32)
        for p in range(NPAIR):
            qcols = slice(0 * D + hbase + p * 128, 0 * D + hbase + (p + 1) * 128)
            kcols = slice(1 * D + hbase + p * 128, 1 * D + hbase + (p + 1) * 128)
            wqk[:, (2 * p) * 128 : (2 * p + 1) * 128] = w_qkv[:, qcols]
            wqk[:, (2 * p + 1) * 128 : (2 * p + 2) * 128] = w_qkv[:, kcols]
            bqk[(2 * p) * 128 : (2 * p + 1) * 128] = b_qkv[qcols]
            bqk[(2 * p + 1) * 128 : (2 * p + 2) * 128] = b_qkv[kcols]
        nmt = 2 * HL * HD // 128
        bqk_col = np.ascontiguousarray(bqk.reshape(nmt, 128).T)  # [128, nmt]

        vcols = slice(2 * D + hbase, 2 * D + hbase + HL * HD)
        wv = np.empty((D + 1, HL * HD), dtype=np.float32)
        wv[:D] = w_qkv[:, vcols]
        wv[D] = b_qkv[vcols]

        wo = np.empty((HL * HD + 1, D), dtype=np.float32)
        wo[: HL * HD] = w_out[hbase : hbase + HL * HD, :]
        wo[HL * HD] = b_out if g == 0 else 0.0

        in_maps.append(
            {
                "xt": np.ascontiguousarray(x[b, :t].T),
                "wqk": wqk,
                "bqk": bqk_col,
                "wv": wv,
                "wo": wo,
            }
        )
    return in_maps


def kernel(x, w_qkv, b_qkv, w_out, b_out):
    x = np.asarray(x, dtype=np.float32)
    w_qkv = np.asarray(w_qkv, dtype=np.float32)
    b_qkv = np.asarray(b_qkv, dtype=np.float32)
    w_out = np.asarray(w_out, dtype=np.float32)
    b_out = np.asarray(b_out, dtype=np.float32)

    nc = build_nc()
    _split_multi_waits(nc)
    in_maps = shard_inputs(x, w_qkv, b_qkv, w_out, b_out)
    res = run_bass_kernel_spmd(nc, in_maps, list(range(NCORES)))
    parts = [np.asarray(res.results[i]["out"]) for i in range(NCORES)]
    out = np.stack([parts[2 * b] + parts[2 * b + 1] for b in range(B)], axis=0)
    return out.astype(np.float32)


# revision 14
# speedup vs baseline: 1.0300x; 1.0300x over previous
"""Trainium2 Bass kernel: multi-head attention (B=4, T=2048, D=768, H=12).

Sharding: 8 cores = 4 batches x 2 head-groups (6 heads each).
Each core computes QKV projection (its heads), attention, and a partial
output projection (contraction over its 384 of 768 w_out rows).
Host unshard: out[b] = partial[2b] + partial[2b+1] (bias folded on core g=0).

Per-core dataflow (everything stays on-chip between input DMA and output DMA):
  - host supplies x[b] pre-transposed: xt [768, 2048]
  - Q^T/K^T computed in transposed layout (pair-packed [128, T] tiles);
    K^T stored per-head zero-padded so every matmul runs in 128-row mode
  - V computed in natural layout with a ones column per head (V~ [tok, 65])
    so the P@V matmul also produces softmax denominators
  - attention in S^T layout: S^T = K^T.T @ Q^T, P^T = exp(S^T/8) via ScalarE
    (fused PSUM eviction), attnU^T = V~.T @ P^T accumulated in PSUM.
    No max-subtraction (scores for this input distribution are within ~[-2.5, 2.8])
    and no transposes of P anywhere.
  - normalize: reciprocal of denominator row into a persistent zero-padded
    row tile, broadcast to 64 partitions with a rank-1 matmul against
    ones_pad (row 0 ones, rest zeros), multiply on DVE into out-proj lhsT layout
  - out-proj from attnN^T pair tiles; b_out via the zero-padded ones k-tile
  - matmuls run in float32r (1 cycle/row at N>=256 vs 4 for plain fp32;
    measured ~1.5e-4 matmul rel err vs 2.4e-3 for bf16). float32r operands
    must come from producers typed float32r, and memset cannot write f32r,
    so constant fills go through f32 twins + DVE copies.

This walrus build encodes at most one sync wait per instruction; Tile emits
several. _split_multi_waits() rewrites the final module, hoisting extra waits
onto same-engine nops inserted right before the offending instruction.
"""

import numpy as np

import concourse.bass as bass
import concourse.mybir as mybir
from concourse.tile import TileContext
from concourse.bass_utils import run_bass_kernel_spmd

# problem constants (fixed by the graded nn.Module)
B, T, D = 4, 2048, 768
H, HD = 12, 64
NCORES = 8
HL = H // 2            # heads per core (2 head-groups)
NPAIR = HL // 2        # head pairs per core

F32 = mybir.dt.float32
F32R = mybir.dt.float32r
BF16 = mybir.dt.bfloat16


def _patch_tile_drain():
    """Kept for API compatibility; the real fix is _split_multi_waits."""


def _split_multi_waits(nc):
    """Walrus here encodes only one sync wait per instruction. Move extra
    waits onto same-engine nops placed immediately before the instruction."""
    n = 0
    for f in nc.m.functions:
        for bb in f.blocks:
            new = []
            for inst in bb.instructions:
                si = inst.sync_info
                if si is not None and si.on_wait and len(si.on_wait) > 1:
                    extra = list(si.on_wait[:-1])
                    keep = si.on_wait[-1]
                    del si.on_wait[:]
                    si.on_wait.append(keep)
                    for w in extra:
                        nop = mybir.InstNoOp(name=f"I-wsplit-{n}", ins=[], outs=[])
                        n += 1
                        nop.engine = inst.engine
                        nop.sync_info = mybir.SyncInfo(on_wait=[w], on_update=[])
                        new.append(nop)
                new.append(inst)
            bb.instructions[:] = new
    return n


def build_nc(t=T, qc=1024, nch=512):
    """Build the SPMD per-core program. t = sequence length, qc = attention
    query chunk (PSUM-limited), nch = matmul moving-dim chunk."""
    tokt = t // 128            # token tiles
    nqc = t // qc              # query chunks
    dk = D // 128              # contraction tiles over D

    nc = bass.Bass("TRN2", target_bir_lowering=False, debug=False)

    # f32r-typed DRAM inputs: the host's fp32 bits reinterpret fine and the
    # PE rounds internally; this satisfies walrus's f32r provenance check.
    xt_d = nc.dram_tensor("xt", [D, t], F32R, kind="ExternalInput")
    wqk_d = nc.dram_tensor("wqk", [D, 2 * HL * HD], F32R, kind="ExternalInput")
    bqk_d = nc.dram_tensor("bqk", [128, 2 * HL * HD // 128], F32, kind="ExternalInput")
    wv_d = nc.dram_tensor("wv", [D + 1, HL * HD], F32R, kind="ExternalInput")
    wo_d = nc.dram_tensor("wo", [HL * HD + 1, D], F32R, kind="ExternalInput")
    out_d = nc.dram_tensor("out", [t, D], F32, kind="ExternalOutput")

    nmt = 2 * HL * HD // 128   # QK projection M-tiles (6)

    def MM(out, lhsT, rhs, start, stop):
        nc.tensor.matmul(out, lhsT, rhs, start=start, stop=stop)

    with TileContext(nc) as tc:
        lp = nc.allow_low_precision(reason="float32r matmul operand production")
        lp.__enter__()
        with tc.tile_pool(name="persist", bufs=1) as pp:
            # ones_pad: row 0 = 1.0, rows 1:128 = 0.0 (bias k-tile / broadcast lhsT)
            ones_pad = pp.tile([128, t], F32R, name="ones_pad")
            QT = [pp.tile([128, t], BF16, name=f"qt{p}") for p in range(NPAIR)]
            KT = [pp.tile([128, t], BF16, name=f"kt{h}") for h in range(HL)]
            V6 = [pp.tile([128, HL * (HD + 1)], BF16, name=f"v6_{c}") for c in range(tokt)]
            WOb = pp.tile([128, D], F32R, name="wob")
            bqk_t = pp.tile([128, nmt], F32, name="bqk_t")
            nc.sync.dma_start(out=bqk_t[:], in_=bqk_d[:, :])

            # ---------------- phase 1: projections ----------------
            with tc.tile_pool(name="phase1", bufs=1) as p1:
                # f32 constant sources (memset can't write f32r)
                ones32 = p1.tile([128, t], F32, name="ones32")
                nc.vector.memset(ones32[:], 0.0)
                nc.vector.memset(ones32[0:1, :], 1.0)
                nc.vector.tensor_copy(ones_pad[:], ones32[:])
                z32 = p1.tile([128, D], F32, name="z32")
                nc.vector.memset(z32[:], 0.0)

                nc.vector.tensor_copy(WOb[:], z32[:])
                nc.sync.dma_start(out=WOb[0:1, :], in_=wo_d[HL * HD : HL * HD + 1, :])

                # zero-pad the complement half of each per-head K^T tile
                for h in range(HL):
                    if h % 2 == 0:
                        nc.vector.memset(KT[h][64:128, :], 0.0)
                    else:
                        nc.vector.memset(KT[h][0:64, :], 0.0)

                xt_t = [p1.tile([128, t], F32R, name=f"x{k}") for k in range(dk)]
                for k in range(dk):
                    nc.sync.dma_start(out=xt_t[k][:], in_=xt_d[k * 128 : (k + 1) * 128, :])
                wqk_t = [p1.tile([128, 2 * HL * HD], F32R, name=f"wqk{k}") for k in range(dk)]
                for k in range(dk):
                    nc.sync.dma_start(
                        out=wqk_t[k][:], in_=wqk_d[k * 128 : (k + 1) * 128, :]
                    )
                wv_t = [p1.tile([128, HL * HD], F32R, name=f"wv{k}") for k in range(dk)]
                for k in range(dk):
                    nc.sync.dma_start(out=wv_t[k][:], in_=wv_d[k * 128 : (k + 1) * 128, :])
                wvb = p1.tile([128, HL * HD], F32R, name="wvb")
                nc.vector.tensor_copy(wvb[:], z32[:, 0 : HL * HD])
                nc.sync.dma_start(out=wvb[0:1, :], in_=wv_d[D : D + 1, :])

                with tc.tile_pool(name="psum_proj", bufs=2, space="PSUM") as prj:
                    # QK^T projection: M-tile 2p = q-pair p, 2p+1 = k-pair p
                    for p in range(NPAIR):
                        for m in (2 * p, 2 * p + 1):
                            for c in range(t // nch):
                                ps = prj.tile([128, nch], F32, tag="qk", bufs=2, name="psqk")
                                for k in range(dk):
                                    MM(
                                        ps[:],
                                        wqk_t[k][:, m * 128 : (m + 1) * 128],
                                        xt_t[k][:, c * nch : (c + 1) * nch],
                                        start=(k == 0),
                                        stop=(k == dk - 1),
                                    )
                                sl = slice(c * nch, (c + 1) * nch)
                                if m == 2 * p:
                                    nc.vector.tensor_scalar_add(
                                        QT[p][:, sl], ps[:], bqk_t[:, m : m + 1]
                                    )
                                else:
                                    h0, h1 = 2 * p, 2 * p + 1
                                    nc.vector.tensor_scalar_add(
                                        KT[h0][0:64, sl], ps[0:64, :], bqk_t[0:64, m : m + 1]
                                    )
                                    nc.vector.tensor_scalar_add(
                                        KT[h1][64:128, sl], ps[64:128, :], bqk_t[64:128, m : m + 1]
                                    )
                    # V projection (natural layout, scattered into V~ tiles)
                    for c in range(tokt):
                        tsl = slice(c * 128, (c + 1) * 128)
                        psv = prj.tile([128, HL * HD], F32, tag="v", bufs=2, name="psv")
                        for k in range(dk):
                            MM(
                                psv[:],
                                xt_t[k][:, tsl],
                                wv_t[k][:],
                                start=(k == 0),
                                stop=False,
                            )
                        MM(psv[:], ones_pad[:, tsl], wvb[:], start=False, stop=True)
                        v = V6[c]
                        v3 = v[:].rearrange("p (h c) -> p h c", c=HD + 1)
                        nc.vector.memset(v3[:, :, HD : HD + 1], 1.0)
                        nc.vector.tensor_copy(
                            v3[:, :, 0:HD],
                            psv[:].rearrange("p (h c) -> p h c", c=HD),
                        )

            # ---------------- phases 2+3 ----------------
            with tc.tile_pool(name="persist2", bufs=1) as pp2:
                AN = [pp2.tile([128, t], F32R, name=f"an{p}") for p in range(NPAIR)]
                WO = [pp2.tile([128, D], F32R, name=f"wop{p}") for p in range(NPAIR)]
                for p in range(NPAIR):
                    nc.sync.dma_start(out=WO[p][:], in_=wo_d[p * 128 : (p + 1) * 128, :])
                # persistent reciprocal row: rows 1:128 zeroed exactly once
                r_pad = pp2.tile([128, qc], F32R, name="r_pad")
                zr32 = pp2.tile([128, qc], F32, name="zr32")
                nc.vector.memset(zr32[:], 0.0)
                nc.vector.tensor_copy(r_pad[:], zr32[:])

                # ---------------- phase 2: attention ----------------
                with (
                    tc.tile_pool(name="psum_s", bufs=2, space="PSUM") as s_pool,
                    tc.tile_pool(name="psum_u", bufs=2, space="PSUM") as u_pool,
                    tc.tile_pool(name="ptp", bufs=3) as ptp,
                ):
                    for p in range(NPAIR):
                        for j in range(2):
                            h = 2 * p + j
                            for q in range(nqc):
                                qsl = slice(q * qc, (q + 1) * qc)
                                au = u_pool.tile([65, qc], F32, tag="au", bufs=2, name="au")
                                for kb in range(tokt):
                                    st = s_pool.tile([128, qc], F32, tag="st", bufs=2, name="st")
                                    for c in range(qc // nch):
                                        MM(
                                            st[:, c * nch : (c + 1) * nch],
                                            KT[h][:, kb * 128 : (kb + 1) * 128],
                                            QT[p][:, q * qc + c * nch : q * qc + (c + 1) * nch],
                                            start=True,
                                            stop=True,
                                        )
                                    pt = ptp.tile([128, qc], BF16, tag="pt", bufs=3, name="pt")
                                    nc.scalar.activation(
                                        pt[:], st[:], mybir.ActivationFunctionType.Exp, scale=0.125
                                    )
                                    for c in range(qc // nch):
                                        MM(
                                            au[:, c * nch : (c + 1) * nch],
                                            V6[kb][:, h * (HD + 1) : (h + 1) * (HD + 1)],
                                            pt[:, c * nch : (c + 1) * nch],
                                            start=(kb == 0),
                                            stop=(kb == tokt - 1),
                                        )
                                # normalize: r = 1/denom into row 0 of r_pad,
                                # rank-1 broadcast matmul, multiply into AN
                                nc.vector.reciprocal(r_pad[0:1, :], au[64:65, :])
                                R = s_pool.tile([128, qc], F32, tag="st", bufs=2, name="R")
                                for c in range(qc // nch):
                                    MM(
                                        R[:, c * nch : (c + 1) * nch],
                                        ones_pad[:, 0:128],
                                        r_pad[:, c * nch : (c + 1) * nch],
                                        start=True,
                                        stop=True,
                                    )
                                R_sb = ptp.tile([64, qc], F32, tag="rsb", bufs=2, name="R_sb")
                                nc.vector.tensor_copy(R_sb[:], R[0:64, :])
                                nc.vector.tensor_mul(
                                    AN[p][j * 64 : (j + 1) * 64, qsl], au[0:64, :], R_sb[:]
                                )

                # ---------------- phase 3: output projection ----------------
                with (
                    tc.tile_pool(name="psum_o", bufs=2, space="PSUM") as o_pool,
                    tc.tile_pool(name="ost", bufs=3) as osp,
                ):
                    for c in range(tokt):
                        tsl = slice(c * 128, (c + 1) * 128)
                        ps = o_pool.tile([128, D], F32, tag="o", bufs=2, name="pso")
                        for n0 in range(0, D, nch):
                            nsz = min(nch, D - n0)
                            nsl = slice(n0, n0 + nsz)
                            for p in range(NPAIR):
                                MM(
                                    ps[:, nsl],
                                    AN[p][:, tsl],
                                    WO[p][:, nsl],
                                    start=(p == 0),
                                    stop=False,
                                )
                            MM(ps[:, nsl], ones_pad[:, tsl], WOb[:, nsl], start=False, stop=True)
                        so = osp.tile([128, D], F32, tag="so", bufs=3, name="so")
                        nc.vector.tensor_copy(so[:], ps[:])
                        nc.sync.dma_start(out=out_d[tsl, :], in_=so[:])
        lp.__exit__(None, None, None)

    return nc


def shard_inputs(x, w_qkv, b_qkv, w_out, b_out, t=T):
    """Build the 8 per-core input maps. Core = (batch, head-group)."""
    in_maps = []
    for core in range(NCORES):
        b, g = divmod(core, 2)
        hbase = HL * g * HD          # first qk column of this group (384*g)
        # q cols then k cols, pair-interleaved: M-tile 2p = q of heads (2p,2p+1),
        # M-tile 2p+1 = k of the same heads.
        wqk = np.empty((D, 2 * HL * HD), dtype=np.float32)
        bqk = np.empty((2 * HL * HD,), dtype=np.float32)
        for p in range(NPAIR):
            qcols = slice(0 * D + hbase + p * 128, 0 * D + hbase + (p + 1) * 128)
            kcols = slice(1 * D + hbase + p * 128, 1 * D + hbase + (p + 1) * 128)
            wqk[:, (2 * p) * 128 : (2 * p + 1) * 128] = w_qkv[:, qcols]
            wqk[:, (2 * p + 1) * 128 : (2 * p + 2) * 128] = w_qkv[:, kcols]
            bqk[(2 * p) * 128 : (2 * p + 1) * 128] = b_qkv[qcols]
            bqk[(2 * p + 1) * 128 : (2 * p + 2) * 128] = b_qkv[kcols]
        nmt = 2 * HL * HD // 128
        bqk_col = np.ascontiguousarray(bqk.reshape(nmt, 128).T)  # [128, nmt]

        vcols = slice(2 * D + hbase, 2 * D + hbase + HL * HD)
        wv = np.empty((D + 1, HL * HD), dtype=np.float32)
        wv[:D] = w_qkv[:, vcols]
        wv[D] = b_qkv[vcols]

        wo = np.empty((HL * HD + 1, D), dtype=np.float32)
        wo[: HL * HD] = w_out[hbase : hbase + HL * HD, :]
        wo[HL * HD] = b_out if g == 0 else 0.0

        in_maps.append(
            {
                "xt": np.ascontiguousarray(x[b, :t].T),
                "wqk": wqk,
                "bqk": bqk_col,
                "wv": wv,
                "wo": wo,
            }
        )
    return in_maps


def kernel(x, w_qkv, b_qkv, w_out, b_out):
    x = np.asarray(x, dtype=np.float32)
    w_qkv = np.asarray(w_qkv, dtype=np.float32)
    b_qkv = np.asarray(b_qkv, dtype=np.float32)
    w_out = np.asarray(w_out, dtype=np.float32)
    b_out = np.asarray(b_out, dtype=np.float32)

    nc = build_nc()
    _split_multi_waits(nc)
    in_maps = shard_inputs(x, w_qkv, b_qkv, w_out, b_out)
    res = run_bass_kernel_spmd(nc, in_maps, list(range(NCORES)))
    parts = [np.asarray(res.results[i]["out"]) for i in range(NCORES)]
    out = np.stack([parts[2 * b] + parts[2 * b + 1] for b in range(B)], axis=0)
    return out.astype(np.float32)


# revision 16
# speedup vs baseline: 1.0932x; 1.0614x over previous
"""Trainium2 Bass kernel: multi-head attention (B=4, T=2048, D=768, H=12).

Sharding: 8 cores = 4 batches x 2 head-groups (6 heads each).
Each core computes QKV projection (its heads), attention, and a partial
output projection (contraction over its 384 of 768 w_out rows).
Host unshard: out[b] = partial[2b] + partial[2b+1] (bias folded on core g=0).

Per-core dataflow (everything stays on-chip between input DMA and output DMA):
  - host supplies x[b] pre-transposed: xt [768, 2048]
  - Q^T/K^T computed in transposed layout (pair-packed [128, T] tiles);
    K^T stored per-head zero-padded so every matmul runs in 128-row mode
  - V computed in natural layout with a ones column per head (V~ [tok, 65])
    so the P@V matmul also produces softmax denominators
  - attention in S^T layout: S^T = K^T.T @ Q^T, P^T = exp(S^T/8) via ScalarE
    (fused PSUM eviction), attnU^T = V~.T @ P^T accumulated in PSUM.
    No max-subtraction (scores for this input distribution are within ~[-2.5, 2.8])
    and no transposes of P anywhere.
  - normalize: reciprocal of denominator row into a persistent zero-padded
    row tile, broadcast to 64 partitions with a rank-1 matmul against
    ones_pad (row 0 ones, rest zeros), multiply on DVE into out-proj lhsT layout
  - out-proj from attnN^T pair tiles; b_out via the zero-padded ones k-tile
  - matmuls run in float32r (1 cycle/row at N>=256 vs 4 for plain fp32;
    measured ~1.5e-4 matmul rel err vs 2.4e-3 for bf16). float32r operands
    must come from producers typed float32r, and memset cannot write f32r,
    so constant fills go through f32 twins + DVE copies.

This walrus build encodes at most one sync wait per instruction; Tile emits
several. _split_multi_waits() rewrites the final module, hoisting extra waits
onto same-engine nops inserted right before the offending instruction.
"""

import numpy as np

import concourse.bass as bass
import concourse.mybir as mybir
from concourse.tile import TileContext
from concourse.bass_utils import run_bass_kernel_spmd

# problem constants (fixed by the graded nn.Module)
B, T, D = 4, 2048, 768
H, HD = 12, 64
NCORES = 8
HL = H // 2            # heads per core (2 head-groups)
NPAIR = HL // 2        # head pairs per core

F32 = mybir.dt.float32
F32R = mybir.dt.float32r
BF16 = mybir.dt.bfloat16


def _patch_tile_drain():
    """Kept for API compatibility; the real fix is _split_multi_waits."""


def _split_multi_waits(nc):
    """Walrus here encodes only one sync wait per instruction. Move extra
    waits onto same-engine nops placed immediately before the instruction."""
    n = 0
    for f in nc.m.functions:
        for bb in f.blocks:
            new = []
            for inst in bb.instructions:
                si = inst.sync_info
                if si is not None and si.on_wait and len(si.on_wait) > 1:
                    extra = list(si.on_wait[:-1])
                    keep = si.on_wait[-1]
                    del si.on_wait[:]
                    si.on_wait.append(keep)
                    for w in extra:
                        nop = mybir.InstNoOp(name=f"I-wsplit-{n}", ins=[], outs=[])
                        n += 1
                        nop.engine = inst.engine
                        nop.sync_info = mybir.SyncInfo(on_wait=[w], on_update=[])
                        new.append(nop)
                new.append(inst)
            bb.instructions[:] = new
    return n


def build_nc(t=T, qc=1024, nch=512):
    """Build the SPMD per-core program. t = sequence length, qc = attention
    query chunk (PSUM-limited), nch = matmul moving-dim chunk."""
    tokt = t // 128            # token tiles
    nqc = t // qc              # query chunks
    dk = D // 128              # contraction tiles over D

    nc = bass.Bass("TRN2", target_bir_lowering=False, debug=False)

    # f32r-typed DRAM inputs: the host's fp32 bits reinterpret fine and the
    # PE rounds internally; this satisfies walrus's f32r provenance check.
    xt_d = nc.dram_tensor("xt", [D, t], F32R, kind="ExternalInput")
    wqk_d = nc.dram_tensor("wqk", [D, 2 * HL * HD], F32R, kind="ExternalInput")
    bqk_d = nc.dram_tensor("bqk", [128, 2 * HL * HD // 128], F32, kind="ExternalInput")
    wv_d = nc.dram_tensor("wv", [D + 1, HL * HD], F32R, kind="ExternalInput")
    wo_d = nc.dram_tensor("wo", [HL * HD + 1, D], F32R, kind="ExternalInput")
    out_d = nc.dram_tensor("out", [t, D], F32, kind="ExternalOutput")

    nmt = 2 * HL * HD // 128   # QK projection M-tiles (6)

    def MM(out, lhsT, rhs, start, stop):
        nc.tensor.matmul(out, lhsT, rhs, start=start, stop=stop)

    with TileContext(nc) as tc:
        lp = nc.allow_low_precision(reason="float32r matmul operand production")
        lp.__enter__()
        with tc.tile_pool(name="persist", bufs=1) as pp:
            # ones_pad: row 0 = 1.0, rows 1:128 = 0.0 (bias k-tile / broadcast lhsT)
            ones_pad = pp.tile([128, t], F32R, name="ones_pad")
            QT = [pp.tile([128, t], BF16, name=f"qt{p}") for p in range(NPAIR)]
            KT = [pp.tile([128, t], BF16, name=f"kt{h}") for h in range(HL)]
            V6 = [pp.tile([128, HL * (HD + 1)], BF16, name=f"v6_{c}") for c in range(tokt)]
            WOb = pp.tile([128, D], F32R, name="wob")
            bqk_t = pp.tile([128, nmt], F32, name="bqk_t")
            nc.sync.dma_start(out=bqk_t[:], in_=bqk_d[:, :])

            # ---------------- phase 1: projections ----------------
            with tc.tile_pool(name="phase1", bufs=1) as p1:
                # f32 constant sources (memset can't write f32r)
                ones32 = p1.tile([128, t], F32, name="ones32")
                nc.vector.memset(ones32[:], 0.0)
                nc.vector.memset(ones32[0:1, :], 1.0)
                nc.vector.tensor_copy(ones_pad[:], ones32[:])
                z32 = p1.tile([128, D], F32, name="z32")
                nc.vector.memset(z32[:], 0.0)

                nc.vector.tensor_copy(WOb[:], z32[:])
                nc.sync.dma_start(out=WOb[0:1, :], in_=wo_d[HL * HD : HL * HD + 1, :])

                # zero-pad the complement half of each per-head K^T tile
                for h in range(HL):
                    if h % 2 == 0:
                        nc.vector.memset(KT[h][64:128, :], 0.0)
                    else:
                        nc.vector.memset(KT[h][0:64, :], 0.0)

                xt_t = [p1.tile([128, t], F32R, name=f"x{k}") for k in range(dk)]
                for k in range(dk):
                    nc.sync.dma_start(out=xt_t[k][:], in_=xt_d[k * 128 : (k + 1) * 128, :])
                wqk_t = [p1.tile([128, 2 * HL * HD], F32R, name=f"wqk{k}") for k in range(dk)]
                for k in range(dk):
                    nc.sync.dma_start(
                        out=wqk_t[k][:], in_=wqk_d[k * 128 : (k + 1) * 128, :]
                    )
                wv_t = [p1.tile([128, HL * HD], F32R, name=f"wv{k}") for k in range(dk)]
                for k in range(dk):
                    nc.sync.dma_start(out=wv_t[k][:], in_=wv_d[k * 128 : (k + 1) * 128, :])
                wvb = p1.tile([128, HL * HD], F32R, name="wvb")
                nc.vector.tensor_copy(wvb[:], z32[:, 0 : HL * HD])
                nc.sync.dma_start(out=wvb[0:1, :], in_=wv_d[D : D + 1, :])

                with tc.tile_pool(name="psum_proj", bufs=2, space="PSUM") as prj:
                    # QK^T projection: M-tile 2p = q-pair p, 2p+1 = k-pair p
                    for p in range(NPAIR):
                        for m in (2 * p, 2 * p + 1):
                            for c in range(t // nch):
                                ps = prj.tile([128, nch], F32, tag="qk", bufs=2, name="psqk")
                                for k in range(dk):
                                    MM(
                                        ps[:],
                                        wqk_t[k][:, m * 128 : (m + 1) * 128],
                                        xt_t[k][:, c * nch : (c + 1) * nch],
                                        start=(k == 0),
                                        stop=(k == dk - 1),
                                    )
                                sl = slice(c * nch, (c + 1) * nch)
                                if m == 2 * p:
                                    nc.vector.tensor_scalar_add(
                                        QT[p][:, sl], ps[:], bqk_t[:, m : m + 1]
                                    )
                                else:
                                    h0, h1 = 2 * p, 2 * p + 1
                                    nc.vector.tensor_scalar_add(
                                        KT[h0][0:64, sl], ps[0:64, :], bqk_t[0:64, m : m + 1]
                                    )
                                    nc.vector.tensor_scalar_add(
                                        KT[h1][64:128, sl], ps[64:128, :], bqk_t[64:128, m : m + 1]
                                    )
                    # V projection (natural layout, scattered into V~ tiles)
                    for c in range(tokt):
                        tsl = slice(c * 128, (c + 1) * 128)
                        psv = prj.tile([128, HL * HD], F32, tag="v", bufs=2, name="psv")
                        for k in range(dk):
                            MM(
                                psv[:],
                                xt_t[k][:, tsl],
                                wv_t[k][:],
                                start=(k == 0),
                                stop=False,
                            )
                        MM(psv[:], ones_pad[:, tsl], wvb[:], start=False, stop=True)
                        v = V6[c]
                        v3 = v[:].rearrange("p (h c) -> p h c", c=HD + 1)
                        nc.vector.memset(v3[:, :, HD : HD + 1], 1.0)
                        nc.vector.tensor_copy(
                            v3[:, :, 0:HD],
                            psv[:].rearrange("p (h c) -> p h c", c=HD),
                        )

            # ---------------- phases 2+3 ----------------
            with tc.tile_pool(name="persist2", bufs=1) as pp2:
                AN = [pp2.tile([128, t], F32R, name=f"an{p}") for p in range(NPAIR)]
                WO = [pp2.tile([128, D], F32R, name=f"wop{p}") for p in range(NPAIR)]
                for p in range(NPAIR):
                    nc.sync.dma_start(out=WO[p][:], in_=wo_d[p * 128 : (p + 1) * 128, :])
                # persistent reciprocal rows (two, alternating per unit so the
                # deferred normalize of unit n-1 never races unit n's write);
                # rows 1:128 zeroed exactly once
                r_pads = [pp2.tile([128, qc], F32R, name=f"r_pad{i}") for i in range(2)]
                zr32 = pp2.tile([128, qc], F32, name="zr32")
                nc.vector.memset(zr32[:], 0.0)
                for i in range(2):
                    nc.vector.tensor_copy(r_pads[i][:], zr32[:])

                # ---------------- phase 2: attention ----------------
                with (
                    tc.tile_pool(name="psum_s", bufs=2, space="PSUM") as s_pool,
                    tc.tile_pool(name="psum_u", bufs=2, space="PSUM") as u_pool,
                    tc.tile_pool(name="ptp", bufs=3) as ptp,
                ):
                    # deferred normalize: unit n's R-matmul/copy/mul are emitted
                    # after unit n+1's attention matmuls so the PE never waits
                    # on the reciprocal chain.
                    pending = None

                    def finish_unit(u):
                        up, uj, uq, uau, urp = u
                        uqsl = slice(uq * qc, (uq + 1) * qc)
                        R = s_pool.tile([128, qc], F32, tag="st", bufs=2, name="R")
                        for c in range(qc // nch):
                            MM(
                                R[:, c * nch : (c + 1) * nch],
                                ones_pad[:, 0:128],
                                urp[:, c * nch : (c + 1) * nch],
                                start=True,
                                stop=True,
                            )
                        R_sb = ptp.tile([64, qc], F32, tag="rsb", bufs=2, name="R_sb")
                        nc.vector.tensor_copy(R_sb[:], R[0:64, :])
                        nc.vector.tensor_mul(
                            AN[up][uj * 64 : (uj + 1) * 64, uqsl], uau[0:64, :], R_sb[:]
                        )

                    unit_no = 0
                    for p in range(NPAIR):
                        for j in range(2):
                            h = 2 * p + j
                            for q in range(nqc):
                                qsl = slice(q * qc, (q + 1) * qc)
                                au = u_pool.tile([65, qc], F32, tag="au", bufs=2, name="au")
                                for kb in range(tokt):
                                    st = s_pool.tile([128, qc], F32, tag="st", bufs=2, name="st")
                                    for c in range(qc // nch):
                                        MM(
                                            st[:, c * nch : (c + 1) * nch],
                                            KT[h][:, kb * 128 : (kb + 1) * 128],
                                            QT[p][:, q * qc + c * nch : q * qc + (c + 1) * nch],
                                            start=True,
                                            stop=True,
                                        )
                                    pt = ptp.tile([128, qc], BF16, tag="pt", bufs=3, name="pt")
                                    nc.scalar.activation(
                                        pt[:], st[:], mybir.ActivationFunctionType.Exp, scale=0.125
                                    )
                                    for c in range(qc // nch):
                                        MM(
                                            au[:, c * nch : (c + 1) * nch],
                                            V6[kb][:, h * (HD + 1) : (h + 1) * (HD + 1)],
                                            pt[:, c * nch : (c + 1) * nch],
                                            start=(kb == 0),
                                            stop=(kb == tokt - 1),
                                        )
                                # finish the previous unit first (its reciprocal
                                # completed during this unit's attention), then
                                # start this unit's reciprocal on the DVE.
                                if pending is not None:
                                    finish_unit(pending)
                                rp_t = r_pads[unit_no % 2]
                                nc.vector.reciprocal(rp_t[0:1, :], au[64:65, :])
                                pending = (p, j, q, au, rp_t)
                                unit_no += 1
                    if pending is not None:
                        finish_unit(pending)

                # ---------------- phase 3: output projection ----------------
                with (
                    tc.tile_pool(name="psum_o", bufs=2, space="PSUM") as o_pool,
                    tc.tile_pool(name="ost", bufs=3) as osp,
                ):
                    for c in range(tokt):
                        tsl = slice(c * 128, (c + 1) * 128)
                        ps = o_pool.tile([128, D], F32, tag="o", bufs=2, name="pso")
                        for n0 in range(0, D, nch):
                            nsz = min(nch, D - n0)
                            nsl = slice(n0, n0 + nsz)
                            for p in range(NPAIR):
                                MM(
                                    ps[:, nsl],
                                    AN[p][:, tsl],
                                    WO[p][:, nsl],
                                    start=(p == 0),
                                    stop=False,
                                )
                            MM(ps[:, nsl], ones_pad[:, tsl], WOb[:, nsl], start=False, stop=True)
                        so = osp.tile([128, D], F32, tag="so", bufs=3, name="so")
                        nc.vector.tensor_copy(so[:], ps[:])
                        nc.sync.dma_start(out=out_d[tsl, :], in_=so[:])
        lp.__exit__(None, None, None)

    return nc


def shard_inputs(x, w_qkv, b_qkv, w_out, b_out, t=T):
    """Build the 8 per-core input maps. Core = (batch, head-group)."""
    in_maps = []
    for core in range(NCORES):
        b, g = divmod(core, 2)
        hbase = HL * g * HD          # first qk column of this group (384*g)
        # q cols then k cols, pair-interleaved: M-tile 2p = q of heads (2p,2p+1),
        # M-tile 2p+1 = k of the same heads.
        wqk = np.empty((D, 2 * HL * HD), dtype=np.float32)
        bqk = np.empty((2 * HL * HD,), dtype=np.float32)
        for p in range(NPAIR):
            qcols = slice(0 * D + hbase + p * 128, 0 * D + hbase + (p + 1) * 128)
            kcols = slice(1 * D + hbase + p * 128, 1 * D + hbase + (p + 1) * 128)
            wqk[:, (2 * p) * 128 : (2 * p + 1) * 128] = w_qkv[:, qcols]
            wqk[:, (2 * p + 1) * 128 : (2 * p + 2) * 128] = w_qkv[:, kcols]
            bqk[(2 * p) * 128 : (2 * p + 1) * 128] = b_qkv[qcols]
            bqk[(2 * p + 1) * 128 : (2 * p + 2) * 128] = b_qkv[kcols]
        nmt = 2 * HL * HD // 128
        bqk_col = np.ascontiguousarray(bqk.reshape(nmt, 128).T)  # [128, nmt]

        vcols = slice(2 * D + hbase, 2 * D + hbase + HL * HD)
        wv = np.empty((D + 1, HL * HD), dtype=np.float32)
        wv[:D] = w_qkv[:, vcols]
        wv[D] = b_qkv[vcols]

        wo = np.empty((HL * HD + 1, D), dtype=np.float32)
        wo[: HL * HD] = w_out[hbase : hbase + HL * HD, :]
        wo[HL * HD] = b_out if g == 0 else 0.0

        in_maps.append(
            {
                "xt": np.ascontiguousarray(x[b, :t].T),
                "wqk": wqk,
                "bqk": bqk_col,
                "wv": wv,
                "wo": wo,
            }
        )
    return in_maps


def kernel(x, w_qkv, b_qkv, w_out, b_out):
    x = np.asarray(x, dtype=np.float32)
    w_qkv = np.asarray(w_qkv, dtype=np.float32)
    b_qkv = np.asarray(b_qkv, dtype=np.float32)
    w_out = np.asarray(w_out, dtype=np.float32)
    b_out = np.asarray(b_out, dtype=np.float32)

    nc = build_nc()
    _split_multi_waits(nc)
    in_maps = shard_inputs(x, w_qkv, b_qkv, w_out, b_out)
    res = run_bass_kernel_spmd(nc, in_maps, list(range(NCORES)))
    parts = [np.asarray(res.results[i]["out"]) for i in range(NCORES)]
    out = np.stack([parts[2 * b] + parts[2 * b + 1] for b in range(B)], axis=0)
    return out.astype(np.float32)


# revision 17
# speedup vs baseline: 1.2969x; 1.1863x over previous
"""Trainium2 Bass kernel: multi-head attention (B=4, T=2048, D=768, H=12).

Sharding: 8 cores = 4 batches x 2 head-groups (6 heads each).
Each core computes QKV projection (its heads), attention, and a partial
output projection (contraction over its 384 of 768 w_out rows).
Host unshard: out[b] = partial[2b] + partial[2b+1] (bias folded on core g=0).

Per-core dataflow (everything stays on-chip between input DMA and output DMA):
  - host supplies x[b] pre-transposed: xt [768, 2048]
  - Q^T/K^T computed in transposed layout (pair-packed [128, T] tiles);
    K^T stored per-head zero-padded so every matmul runs in 128-row mode
  - V computed in natural layout with a ones column per head (V~ [tok, 65])
    so the P@V matmul also produces softmax denominators
  - attention in S^T layout: S^T = K^T.T @ Q^T, P^T = exp(S^T/8) via ScalarE
    (fused PSUM eviction), attnU^T = V~.T @ P^T accumulated in PSUM.
    No max-subtraction (scores for this input distribution are within ~[-2.5, 2.8])
    and no transposes of P anywhere.
  - normalize: reciprocal of denominator row into a persistent zero-padded
    row tile, broadcast to 64 partitions with a rank-1 matmul against
    ones_pad (row 0 ones, rest zeros), multiply on DVE into out-proj lhsT layout
  - out-proj from attnN^T pair tiles; b_out via the zero-padded ones k-tile
  - matmuls run in float32r (1 cycle/row at N>=256 vs 4 for plain fp32;
    measured ~1.5e-4 matmul rel err vs 2.4e-3 for bf16). float32r operands
    must come from producers typed float32r, and memset cannot write f32r,
    so constant fills go through f32 twins + DVE copies.

This walrus build encodes at most one sync wait per instruction; Tile emits
several. _split_multi_waits() rewrites the final module, hoisting extra waits
onto same-engine nops inserted right before the offending instruction.
"""

import numpy as np

import concourse.bass as bass
import concourse.mybir as mybir
from concourse.tile import TileContext
from concourse.bass_utils import run_bass_kernel_spmd

# problem constants (fixed by the graded nn.Module)
B, T, D = 4, 2048, 768
H, HD = 12, 64
NCORES = 8
HL = H // 2            # heads per core (2 head-groups)
NPAIR = HL // 2        # head pairs per core

F32 = mybir.dt.float32
F32R = mybir.dt.float32r
BF16 = mybir.dt.bfloat16


def _patch_tile_drain():
    """Kept for API compatibility; the real fix is _split_multi_waits."""


def _split_multi_waits(nc):
    """Walrus here encodes only one sync wait per instruction. Move extra
    waits onto same-engine nops placed immediately before the instruction."""
    n = 0
    for f in nc.m.functions:
        for bb in f.blocks:
            new = []
            for inst in bb.instructions:
                si = inst.sync_info
                if si is not None and si.on_wait and len(si.on_wait) > 1:
                    extra = list(si.on_wait[:-1])
                    keep = si.on_wait[-1]
                    del si.on_wait[:]
                    si.on_wait.append(keep)
                    for w in extra:
                        nop = mybir.InstNoOp(name=f"I-wsplit-{n}", ins=[], outs=[])
                        n += 1
                        nop.engine = inst.engine
                        nop.sync_info = mybir.SyncInfo(on_wait=[w], on_update=[])
                        new.append(nop)
                new.append(inst)
            bb.instructions[:] = new
    return n


def build_nc(t=T, qc=1024, nch=512):
    """Build the SPMD per-core program. t = sequence length, qc = attention
    query chunk (PSUM-limited), nch = matmul moving-dim chunk."""
    tokt = t // 128            # token tiles
    nqc = t // qc              # query chunks
    dk = D // 128              # contraction tiles over D

    nc = bass.Bass("TRN2", target_bir_lowering=False, debug=False)

    # f32r-typed DRAM inputs: the host's fp32 bits reinterpret fine and the
    # PE rounds internally; this satisfies walrus's f32r provenance check.
    xt_d = nc.dram_tensor("xt", [D, t], F32R, kind="ExternalInput")
    wqk_d = nc.dram_tensor("wqk", [D, 2 * HL * HD], F32R, kind="ExternalInput")
    bqk_d = nc.dram_tensor("bqk", [128, 2 * HL * HD // 128], F32, kind="ExternalInput")
    wv_d = nc.dram_tensor("wv", [D + 1, HL * HD], F32R, kind="ExternalInput")
    wo_d = nc.dram_tensor("wo", [HL * HD + 1, D], F32R, kind="ExternalInput")
    out_d = nc.dram_tensor("out", [t, D], F32, kind="ExternalOutput")

    nmt = 2 * HL * HD // 128   # QK projection M-tiles (6)

    def MM(out, lhsT, rhs, start, stop):
        nc.tensor.matmul(out, lhsT, rhs, start=start, stop=stop)

    with TileContext(nc) as tc:
        lp = nc.allow_low_precision(reason="float32r matmul operand production")
        lp.__enter__()
        with tc.tile_pool(name="persist", bufs=1) as pp:
            # ones_pad: row 0 = 1.0, rows 1:128 = 0.0 (bias k-tile / broadcast lhsT)
            ones_pad = pp.tile([128, t], F32R, name="ones_pad")
            QT = [pp.tile([128, t], BF16, name=f"qt{p}") for p in range(NPAIR)]
            KT = [pp.tile([128, t], BF16, name=f"kt{h}") for h in range(HL)]
            V6 = [pp.tile([128, HL * (HD + 1)], BF16, name=f"v6_{c}") for c in range(tokt)]
            WOb = pp.tile([128, D], F32R, name="wob")
            bqk_t = pp.tile([128, nmt], F32, name="bqk_t")
            nc.sync.dma_start(out=bqk_t[:], in_=bqk_d[:, :])

            # ---------------- phase 1: projections ----------------
            with tc.tile_pool(name="phase1", bufs=1) as p1:
                # f32 constant sources (memset can't write f32r)
                ones32 = p1.tile([128, t], F32, name="ones32")
                nc.vector.memset(ones32[:], 0.0)
                nc.vector.memset(ones32[0:1, :], 1.0)
                nc.vector.tensor_copy(ones_pad[:], ones32[:])
                z32 = p1.tile([128, D], F32, name="z32")
                nc.vector.memset(z32[:], 0.0)

                nc.vector.tensor_copy(WOb[:], z32[:])
                nc.sync.dma_start(out=WOb[0:1, :], in_=wo_d[HL * HD : HL * HD + 1, :])

                # zero-pad the complement half of each per-head K^T tile
                for h in range(HL):
                    if h % 2 == 0:
                        nc.vector.memset(KT[h][64:128, :], 0.0)
                    else:
                        nc.vector.memset(KT[h][0:64, :], 0.0)

                xt_t = [p1.tile([128, t], F32R, name=f"x{k}") for k in range(dk)]
                for k in range(dk):
                    nc.sync.dma_start(out=xt_t[k][:], in_=xt_d[k * 128 : (k + 1) * 128, :])
                wqk_t = [p1.tile([128, 2 * HL * HD], F32R, name=f"wqk{k}") for k in range(dk)]
                for k in range(dk):
                    nc.sync.dma_start(
                        out=wqk_t[k][:], in_=wqk_d[k * 128 : (k + 1) * 128, :]
                    )
                wv_t = [p1.tile([128, HL * HD], F32R, name=f"wv{k}") for k in range(dk)]
                for k in range(dk):
                    nc.sync.dma_start(out=wv_t[k][:], in_=wv_d[k * 128 : (k + 1) * 128, :])
                wvb = p1.tile([128, HL * HD], F32R, name="wvb")
                nc.vector.tensor_copy(wvb[:], z32[:, 0 : HL * HD])
                nc.sync.dma_start(out=wvb[0:1, :], in_=wv_d[D : D + 1, :])

                with tc.tile_pool(name="psum_proj", bufs=2, space="PSUM") as prj:
                    # QK^T projection: M-tile 2p = q-pair p, 2p+1 = k-pair p
                    for p in range(NPAIR):
                        for m in (2 * p, 2 * p + 1):
                            for c in range(t // nch):
                                ps = prj.tile([128, nch], F32, tag="qk", bufs=2, name="psqk")
                                for k in range(dk):
                                    MM(
                                        ps[:],
                                        wqk_t[k][:, m * 128 : (m + 1) * 128],
                                        xt_t[k][:, c * nch : (c + 1) * nch],
                                        start=(k == 0),
                                        stop=(k == dk - 1),
                                    )
                                sl = slice(c * nch, (c + 1) * nch)
                                if m == 2 * p:
                                    nc.vector.tensor_scalar_add(
                                        QT[p][:, sl], ps[:], bqk_t[:, m : m + 1]
                                    )
                                else:
                                    h0, h1 = 2 * p, 2 * p + 1
                                    nc.vector.tensor_scalar_add(
                                        KT[h0][0:64, sl], ps[0:64, :], bqk_t[0:64, m : m + 1]
                                    )
                                    nc.vector.tensor_scalar_add(
                                        KT[h1][64:128, sl], ps[64:128, :], bqk_t[64:128, m : m + 1]
                                    )
                    # V projection (natural layout, scattered into V~ tiles)
                    for c in range(tokt):
                        tsl = slice(c * 128, (c + 1) * 128)
                        psv = prj.tile([128, HL * HD], F32, tag="v", bufs=2, name="psv")
                        for k in range(dk):
                            MM(
                                psv[:],
                                xt_t[k][:, tsl],
                                wv_t[k][:],
                                start=(k == 0),
                                stop=False,
                            )
                        MM(psv[:], ones_pad[:, tsl], wvb[:], start=False, stop=True)
                        v = V6[c]
                        v3 = v[:].rearrange("p (h c) -> p h c", c=HD + 1)
                        nc.vector.memset(v3[:, :, HD : HD + 1], 1.0)
                        nc.vector.tensor_copy(
                            v3[:, :, 0:HD],
                            psv[:].rearrange("p (h c) -> p h c", c=HD),
                        )

            # ---------------- phases 2+3 ----------------
            with tc.tile_pool(name="persist2", bufs=1) as pp2:
                AN = [pp2.tile([128, t], F32R, name=f"an{p}") for p in range(NPAIR)]
                WO = [pp2.tile([128, D], F32R, name=f"wop{p}") for p in range(NPAIR)]
                for p in range(NPAIR):
                    nc.sync.dma_start(out=WO[p][:], in_=wo_d[p * 128 : (p + 1) * 128, :])
                # persistent reciprocal rows (two, alternating per unit so the
                # deferred normalize of unit n-1 never races unit n's write);
                # rows 1:128 zeroed exactly once
                r_pads = [pp2.tile([128, qc], F32R, name=f"r_pad{i}") for i in range(2)]
                zr32 = pp2.tile([128, qc], F32, name="zr32")
                nc.vector.memset(zr32[:], 0.0)
                for i in range(2):
                    nc.vector.tensor_copy(r_pads[i][:], zr32[:])

                # ---------------- phase 2: attention ----------------
                with (
                    tc.tile_pool(name="psum_s", bufs=2, space="PSUM") as s_pool,
                    tc.tile_pool(name="psum_u", bufs=2, space="PSUM") as u_pool,
                    tc.tile_pool(name="ptp", bufs=3) as ptp,
                ):
                    # deferred normalize: unit n's R-matmul/copy/mul are emitted
                    # after unit n+1's attention matmuls so the PE never waits
                    # on the reciprocal chain.
                    pending = None

                    def finish_unit(u):
                        up, uj, uq, uau, urp = u
                        uqsl = slice(uq * qc, (uq + 1) * qc)
                        R = s_pool.tile([128, qc], F32, tag="st", bufs=2, name="R")
                        for c in range(qc // nch):
                            MM(
                                R[:, c * nch : (c + 1) * nch],
                                ones_pad[:, 0:128],
                                urp[:, c * nch : (c + 1) * nch],
                                start=True,
                                stop=True,
                            )
                        R_sb = ptp.tile([64, qc], F32, tag="rsb", bufs=2, name="R_sb")
                        nc.vector.tensor_copy(R_sb[:], R[0:64, :])
                        nc.vector.tensor_mul(
                            AN[up][uj * 64 : (uj + 1) * 64, uqsl], uau[0:64, :], R_sb[:]
                        )

                    unit_no = 0
                    for p in range(NPAIR):
                        for j in range(2):
                            h = 2 * p + j
                            for q in range(nqc):
                                qsl = slice(q * qc, (q + 1) * qc)
                                au = u_pool.tile([65, qc], F32, tag="au", bufs=2, name="au")
                                for kb in range(tokt):
                                    if kb == tokt // 2 and pending is not None:
                                        # finish the previous unit mid-way through
                                        # this one: its reciprocal is done by now and
                                        # the au slot frees well before the next unit
                                        finish_unit(pending)
                                        pending = None
                                    st = s_pool.tile([128, qc], F32, tag="st", bufs=2, name="st")
                                    for c in range(qc // nch):
                                        MM(
                                            st[:, c * nch : (c + 1) * nch],
                                            KT[h][:, kb * 128 : (kb + 1) * 128],
                                            QT[p][:, q * qc + c * nch : q * qc + (c + 1) * nch],
                                            start=True,
                                            stop=True,
                                        )
                                    pt = ptp.tile([128, qc], BF16, tag="pt", bufs=3, name="pt")
                                    nc.scalar.activation(
                                        pt[:], st[:], mybir.ActivationFunctionType.Exp, scale=0.125
                                    )
                                    for c in range(qc // nch):
                                        MM(
                                            au[:, c * nch : (c + 1) * nch],
                                            V6[kb][:, h * (HD + 1) : (h + 1) * (HD + 1)],
                                            pt[:, c * nch : (c + 1) * nch],
                                            start=(kb == 0),
                                            stop=(kb == tokt - 1),
                                        )
                                if pending is not None:
                                    finish_unit(pending)
                                    pending = None
                                rp_t = r_pads[unit_no % 2]
                                nc.vector.reciprocal(rp_t[0:1, :], au[64:65, :])
                                pending = (p, j, q, au, rp_t)
                                unit_no += 1
                    if pending is not None:
                        finish_unit(pending)

                # ---------------- phase 3: output projection ----------------
                with (
                    tc.tile_pool(name="psum_o", bufs=2, space="PSUM") as o_pool,
                    tc.tile_pool(name="ost", bufs=3) as osp,
                ):
                    for c in range(tokt):
                        tsl = slice(c * 128, (c + 1) * 128)
                        ps = o_pool.tile([128, D], F32, tag="o", bufs=2, name="pso")
                        for n0 in range(0, D, nch):
                            nsz = min(nch, D - n0)
                            nsl = slice(n0, n0 + nsz)
                            for p in range(NPAIR):
                                MM(
                                    ps[:, nsl],
                                    AN[p][:, tsl],
                                    WO[p][:, nsl],
                                    start=(p == 0),
                                    stop=False,
                                )
                            MM(ps[:, nsl], ones_pad[:, tsl], WOb[:, nsl], start=False, stop=True)
                        so = osp.tile([128, D], F32, tag="so", bufs=3, name="so")
                        nc.vector.tensor_copy(so[:], ps[:])
                        nc.sync.dma_start(out=out_d[tsl, :], in_=so[:])
        lp.__exit__(None, None, None)

    return nc


def shard_inputs(x, w_qkv, b_qkv, w_out, b_out, t=T):
    """Build the 8 per-core input maps. Core = (batch, head-group)."""
    in_maps = []
    for core in range(NCORES):
        b, g = divmod(core, 2)
        hbase = HL * g * HD          # first qk column of this group (384*g)
        # q cols then k cols, pair-interleaved: M-tile 2p = q of heads (2p,2p+1),
        # M-tile 2p+1 = k of the same heads.
        wqk = np.empty((D, 2 * HL * HD), dtype=np.float32)
        bqk = np.empty((2 * HL * HD,), dtype=np.float32)
        for p in range(NPAIR):
            qcols = slice(0 * D + hbase + p * 128, 0 * D + hbase + (p + 1) * 128)
            kcols = slice(1 * D + hbase + p * 128, 1 * D + hbase + (p + 1) * 128)
            wqk[:, (2 * p) * 128 : (2 * p + 1) * 128] = w_qkv[:, qcols]
            wqk[:, (2 * p + 1) * 128 : (2 * p + 2) * 128] = w_qkv[:, kcols]
            bqk[(2 * p) * 128 : (2 * p + 1) * 128] = b_qkv[qcols]
            bqk[(2 * p + 1) * 128 : (2 * p + 2) * 128] = b_qkv[kcols]
        nmt = 2 * HL * HD // 128
        bqk_col = np.ascontiguousarray(bqk.reshape(nmt, 128).T)  # [128, nmt]

        vcols = slice(2 * D + hbase, 2 * D + hbase + HL * HD)
        wv = np.empty((D + 1, HL * HD), dtype=np.float32)
        wv[:D] = w_qkv[:, vcols]
        wv[D] = b_qkv[vcols]

        wo = np.empty((HL * HD + 1, D), dtype=np.float32)
        wo[: HL * HD] = w_out[hbase : hbase + HL * HD, :]
        wo[HL * HD] = b_out if g == 0 else 0.0

        in_maps.append(
            {
                "xt": np.ascontiguousarray(x[b, :t].T),
                "wqk": wqk,
                "bqk": bqk_col,
                "wv": wv,
                "wo": wo,
            }
        )
    return in_maps


def kernel(x, w_qkv, b_qkv, w_out, b_out):
    x = np.asarray(x, dtype=np.float32)
    w_qkv = np.asarray(w_qkv, dtype=np.float32)
    b_qkv = np.asarray(b_qkv, dtype=np.float32)
    w_out = np.asarray(w_out, dtype=np.float32)
    b_out = np.asarray(b_out, dtype=np.float32)

    nc = build_nc()
    _split_multi_waits(nc)
    in_maps = shard_inputs(x, w_qkv, b_qkv, w_out, b_out)
    res = run_bass_kernel_spmd(nc, in_maps, list(range(NCORES)))
    parts = [np.asarray(res.results[i]["out"]) for i in range(NCORES)]
    out = np.stack([parts[2 * b] + parts[2 * b + 1] for b in range(B)], axis=0)
    return out.astype(np.float32)
